# revision 33
# baseline (speedup 1.0000x reference)
"""Trainium2 Bass kernel for nn_Loss_factory_12429635355015.

Loss = NLLSurv + CohortLoss(intra + inter) over a [4, 8192, 4, 256] cohort bank.

Strategy (memory-bound, 8 NeuronCores):
  - Shard cohort_bank along the N (bank-entry) axis: each core streams its
    16 MiB shard once at HBM line rate (4 tiles x 4 MiB contiguous SWDGE
    cast-DMAs, f32 HBM -> bf16 SBUF). One tile == one class (1024 entries).
  - ALL input DMAs go through the SWDGE queue, small tensors first: while
    the bank stream saturates the DMA engines, HWDGE-queue DMAs are starved
    for tens of us, so nothing else may ride the HWDGE queues mid-kernel.
  - Per 1024-entry tile, work is spread so DMA stays the bottleneck:
      DVE:  3 bf16 adds (component sum S) + broadcast normalize
      POOL: S*S and the per-entry sum-of-squares reduce (+ DMA desc-gens)
      ACT:  rsqrt via Ln+Exp on [128,8]; 2 PSUM->SBUF copies; exp(sims/tau)
            with accum_out giving per-class partial sums directly
      PE :  16 [128x128] transposes (S -> S^T) + 4 matmuls vs anchors
  - NLL + intra terms are computed on-device from host-encoded one-hots
    (index encoding only; all arithmetic on device).
  - Each core outputs [ep_partial, en_partial, nll+intra]; the host sums the
    two scalars across cores (the 'all-reduce two scalars' step) and applies
    the final -log((ep+eps)/(ep+en+eps)).
"""

import math
import os
import sys

import numpy as np

for _p in ("/opt/trn_rl_repo",):
    if _p not in sys.path and os.path.isdir(_p):
        sys.path.insert(0, _p)

import concourse.bacc as bacc
import concourse.tile as tile
from concourse import mybir
from concourse.bass_utils import run_bass_kernel_spmd

# Pin every activation to the one table set that contains all functions this
# kernel uses (Square/Ln/Exp/Copy/Abs/Identity). Without this, Bacc's
# first-match set selection alternates between sets (Ln lives outside the
# default exp set) and reloads the ACT tables ~1.3us per switch every tile.
_ACT_SET = "natural_log_exp_and_others"


def _pin_act_tables():
    import functools
    import concourse.hw_specs as hw_specs
    if getattr(hw_specs.get_activation_tables, "_pinned", False):
        return
    orig = hw_specs.get_activation_tables

    @functools.cache
    def pinned(arch):
        tabs = orig(arch)
        return {k: (v if k == _ACT_SET else set()) for k, v in tabs.items()}

    pinned._pinned = True
    hw_specs.get_activation_tables = pinned
    bacc.get_activation_tables = pinned


_pin_act_tables()

F32 = mybir.dt.float32
AF = mybir.ActivationFunctionType

# Problem constants (hardcoded per spec).
B = 64            # batch
K = 4             # n_cls
C = 256           # feature dim
NB = 8192         # bank entries per class (global)
NCORES = 8
NSH = NB // NCORES          # 1024 bank entries per class per core
ROWS = K * NSH              # 4096 rows of [4*256] per core
NT = 1024                   # bank entries per tile (4 MiB) == one class
TILES = ROWS // NT          # 4
EPG = NT // 128             # 8 entries per partition per tile
EPS_NLL = 1e-7
EPS_COH = 1e-8

# packed small-input layout: [B, SM_W] f32
#   indiv [0:1024) | gp [1024:1536) | haz [1536:1540) | spad [1540:1545)
#   ohy [1545:1550) | ohy1 [1550:1555) | oh4 [1555:1559) | cfs [1559:1561)
SM_W = 1568


def _build():
    nc = bacc.Bacc("TRN2", target_bir_lowering=False, debug=False,
                   enable_asserts=False, num_devices=NCORES)

    bank = nc.dram_tensor("bank", [ROWS, 1024], F32, kind="ExternalInput")
    smalls = nc.dram_tensor("smalls", [B, SM_W], F32, kind="ExternalInput")
    out_d = nc.dram_tensor("out_vec", [4, 1], F32, kind="ExternalOutput")

    import ml_dtypes
    ident_d = nc.inline_tensor(np.eye(128, dtype=np.float32), "ident")
    ident_bf_d = nc.inline_tensor(np.eye(128, dtype=ml_dtypes.bfloat16), "ident_bf")
    ones_d = nc.inline_tensor(np.ones((128, 1), dtype=np.float32), "ones_col")

    v = nc.vector
    a = nc.scalar
    g = nc.gpsimd

    with tile.TileContext(nc) as tc:
        from contextlib import ExitStack
        with ExitStack() as ctx:
            const = ctx.enter_context(tc.tile_pool(name="const", bufs=1))
            small = ctx.enter_context(tc.tile_pool(name="small", bufs=1))
            tpool = ctx.enter_context(tc.tile_pool(name="T", bufs=2))
            spool = ctx.enter_context(tc.tile_pool(name="S", bufs=2))
            stpool = ctx.enter_context(tc.tile_pool(name="STsb", bufs=2))
            epool = ctx.enter_context(tc.tile_pool(name="esb", bufs=2))
            ps_st = ctx.enter_context(tc.tile_pool(name="ps_st", bufs=1, space="PSUM"))
            ps_p = ctx.enter_context(tc.tile_pool(name="ps_p", bufs=1, space="PSUM"))

            BF16 = mybir.dt.bfloat16

            # ---------- all input DMAs on the SWDGE queue, smalls first ----
            sm_sb = small.tile([B, SM_W], F32)
            g.dma_start(out=sm_sb[:], in_=smalls[:])
            ident_sb = const.tile([128, 128], F32)
            g.dma_start(out=ident_sb[:], in_=ident_d[:])
            ident_bf = const.tile([128, 128], BF16)
            g.dma_start(out=ident_bf[:], in_=ident_bf_d[:])
            ones_sb = const.tile([128, 1], F32)
            g.dma_start(out=ones_sb[:], in_=ones_d[:])

            # Chunked streaming: tiles 0-2 as 2 MiB halves, tile 3 (the last
            # class) as 1 MiB quarters so the drain chain after the final DMA
            # byte is as short as possible.  chunk = (tile, e-offset, e-count,
            # accum slot).
            CHUNKS = [(t, q * 4, 4, 2 * t + q)
                      for t in range(3) for q in range(2)]
            CHUNKS += [(3, q * 2, 2, 6 + q) for q in range(4)]
            NCH = len(CHUNKS)
            NSLOT = 10

            T_sb = {}
            for i in range(NCH):
                T_sb[i] = tpool.tile([128, 4 * 1024], BF16,
                                     name="Tsb", tag=f"T{i % 2}")

            def emit_bank_dma(i):
                t, e0, eh, _ = CHUNKS[i]
                src = bank[t * NT:(t + 1) * NT, :].rearrange(
                    "(p e) x -> p e x", e=EPG)[:, e0:e0 + eh, :]
                g.dma_start(
                    out=T_sb[i][:, 0:eh * 1024].rearrange(
                        "p (e x) -> p e x", e=eh),
                    in_=src)

            for i in range(4):
                emit_bank_dma(i)

            # views into the packed small-input tile
            ind_sb = sm_sb[:, 0:1024]
            gp_sb = sm_sb[:, 1024:1536]
            haz_sb = sm_sb[:, 1536:1540]
            spad_sb = sm_sb[:, 1540:1545]
            ohy_sb = sm_sb[:, 1545:1550]
            ohy1_sb = sm_sb[:, 1550:1555]
            oh4_sb = sm_sb[:, 1555:1559]
            cfs_sb = sm_sb[:, 1559:1561]

            # ---------- anchors: A = l2norm(mean_j indiv[b,j,:]) ----------
            iv = ind_sb.rearrange("p (j c) -> p j c", j=4)
            asum = small.tile([B, C], F32)
            atmp = small.tile([B, C], F32)
            v.tensor_add(asum[:], iv[:, 0, :], iv[:, 1, :])
            v.tensor_add(atmp[:], iv[:, 2, :], iv[:, 3, :])
            v.tensor_add(asum[:], asum[:], atmp[:])
            sqa = small.tile([B, C], F32)
            ssa = small.tile([B, 1], F32)
            a.activation(sqa[:], asum[:], AF.Square, accum_out=ssa[:])
            lna = small.tile([B, 1], F32)
            a.activation(lna[:], ssa[:], AF.Ln)
            rsa = small.tile([B, 1], F32)
            a.activation(rsa[:], lna[:], AF.Exp, scale=-0.5)
            v.tensor_scalar_mul(asum[:], asum[:], rsa[:])
            at_ps = ps_p.tile([128, 2, B], F32, tag="p0")
            for h in range(2):
                nc.tensor.transpose(at_ps[:, h, :], asum[:, h * 128:(h + 1) * 128],
                                    ident_sb[0:B, 0:B])
            at_sb = const.tile([128, 2, B], BF16)
            a.copy(at_sb[:], at_ps[:])

            # ---------- NLL (per-b, b on partitions) ----------
            MUL = mybir.AluOpType.mult
            t5 = small.tile([B, K + 1], F32)
            t4 = small.tile([B, K], F32)
            sy = small.tile([B, 1], F32)
            hy = small.tile([B, 1], F32)
            sy1 = small.tile([B, 1], F32)
            v.scalar_tensor_tensor(t5[:], spad_sb[:], 1.0, ohy_sb[:],
                                   MUL, MUL, accum_out=sy[:])
            v.scalar_tensor_tensor(t4[:], haz_sb[:], 1.0, ohy_sb[:, 0:K],
                                   MUL, MUL, accum_out=hy[:])
            v.scalar_tensor_tensor(t5[:], spad_sb[:], 1.0, ohy1_sb[:],
                                   MUL, MUL, accum_out=sy1[:])
            for x in (sy, hy, sy1):
                v.tensor_scalar_max(x[:], x[:], EPS_NLL)
            lsy = small.tile([B, 1], F32)
            lhy = small.tile([B, 1], F32)
            lsy1 = small.tile([B, 1], F32)
            a.activation(lsy[:], sy[:], AF.Ln)
            a.activation(lhy[:], hy[:], AF.Ln)
            a.activation(lsy1[:], sy1[:], AF.Ln)
            tu = small.tile([B, 1], F32)
            tcen = small.tile([B, 1], F32)
            negl = small.tile([B, 1], F32)
            v.tensor_add(tu[:], lsy[:], lhy[:])
            v.tensor_mul(tu[:], tu[:], cfs_sb[:, 1:2])      # *(1-cf)
            v.tensor_mul(tcen[:], lsy1[:], cfs_sb[:, 0:1])  # *cf
            v.tensor_add(negl[:], tu[:], tcen[:])           # = -neg_l per b

            # ---------- intra cohort term ----------
            # dots on raw vectors, then rescale D by rsi[p]*rsg[t] afterward
            prod = small.tile([B, C], F32)
            ssqi = small.tile([B, 4], F32)
            for j in range(4):
                a.activation(sqa[:], iv[:, j, :], AF.Square,
                             accum_out=ssqi[:, j:j + 1])
            rsi = small.tile([B, 4], F32)
            a.activation(rsi[:], ssqi[:], AF.Ln)
            a.activation(rsi[:], rsi[:], AF.Exp, scale=-0.5)
            gv = gp_sb.rearrange("p (t c) -> p t c", t=2)
            ssqg = small.tile([B, 2], F32)
            for tt in range(2):
                a.activation(sqa[:], gv[:, tt, :], AF.Square,
                             accum_out=ssqg[:, tt:tt + 1])
            rsg = small.tile([B, 2], F32)
            a.activation(rsg[:], ssqg[:], AF.Ln)
            a.activation(rsg[:], rsg[:], AF.Exp, scale=-0.5)
            D = small.tile([B, 8], F32)
            for p in range(4):
                for t in range(2):
                    col = p * 2 + t
                    v.scalar_tensor_tensor(prod[:], iv[:, p, :], 1.0,
                                           gv[:, t, :], MUL, MUL,
                                           accum_out=D[:, col:col + 1])
            Dv = D.rearrange("p (j t) -> p j t", t=2)
            v.tensor_mul(Dv[:], Dv[:], rsi[:, :, None].broadcast_to([B, 4, 2]))
            v.tensor_mul(Dv[:], Dv[:], rsg[:, None, :].broadcast_to([B, 4, 2]))
            U = small.tile([B, 8], F32)
            a.activation(U[:], D[:], AF.Abs)
            # mask==1 entries (cols 0,1,4,7) use -sim instead of |sim|
            v.tensor_scalar_mul(U[:, 0:2], D[:, 0:2], -1.0)
            v.tensor_scalar_mul(U[:, 4:5], D[:, 4:5], -1.0)
            v.tensor_scalar_mul(U[:, 7:8], D[:, 7:8], -1.0)
            isum = small.tile([B, 1], F32)
            v.reduce_sum(isum[:], U[:], axis=mybir.AxisListType.X)
            # contrib_b = -negl/B + isum/(8B) + 1/B  -> sums to nll + intra_loss
            c1 = small.tile([B, 1], F32)
            c2 = small.tile([B, 1], F32)
            contrib = small.tile([B, 1], F32)
            v.tensor_scalar_mul(c1[:], negl[:], -1.0 / B)
            v.tensor_scalar_mul(c2[:], isum[:], 1.0 / (8 * B))
            v.tensor_add(contrib[:], c1[:], c2[:])
            v.tensor_scalar_add(contrib[:], contrib[:], 1.0 / B)

            # ---------- main loop over bank chunks ----------
            Eh = small.tile([B, NSLOT], F32)   # accum_out per chunk
            for i in range(NCH):
                t, e0, eh, slot = CHUNKS[i]
                if i + 4 < NCH:
                    emit_bank_dma(i + 4)
                W = eh * 128   # sims columns in this chunk
                Tv = T_sb[i][:, 0:eh * 1024].rearrange(
                    "p (e j c) -> p e j c", e=eh, j=4)
                S_sb = spool.tile([128, 4 * C], BF16, tag=f"S{i % 3}")
                Sv = S_sb[:, 0:eh * C].rearrange("p (e c) -> p e c", e=eh)
                tmp = spool.tile([128, 4 * C], BF16, tag=f"tmp{i % 3}")
                tv = tmp[:, 0:eh * C].rearrange("p (e c) -> p e c", e=eh)
                v.tensor_add(Sv[:], Tv[:, :, 0, :], Tv[:, :, 1, :])
                v.tensor_add(tv[:], Tv[:, :, 2, :], Tv[:, :, 3, :])
                v.tensor_add(Sv[:], Sv[:], tv[:])
                # S*S on the pool engine while DMA streams; on DVE during the
                # post-stream drain (shorter chain)
                sq = spool.tile([128, 4 * C], BF16, tag=f"sq{i % 3}")
                sqv = sq[:, 0:eh * C]
                (v if i >= 8 else g).tensor_mul(sqv[:], S_sb[:, 0:eh * C],
                                                S_sb[:, 0:eh * C])
                ss = spool.tile([128, 4], F32, tag=f"ss{i % 3}")
                v.reduce_sum(ss[:, 0:eh],
                             sqv.rearrange("p (e c) -> p e c", e=eh),
                             axis=mybir.AxisListType.X)
                rr = spool.tile([128, 4], F32, tag=f"rr{i % 3}")
                a.activation(rr[:, 0:eh], ss[:, 0:eh], AF.Ln)
                a.activation(rr[:, 0:eh], rr[:, 0:eh], AF.Exp, scale=-0.5)
                v.tensor_mul(Sv[:], Sv[:],
                             rr[:, 0:eh, None].broadcast_to([128, eh, C]))
                # transpose normalized S into [c, n] chunks (h = c-half)
                st_ps = [ps_st.tile([128, 512], BF16, name="stps",
                                    tag=f"stps{h}{i % 3}")
                         for h in range(2)]
                st_sb = [stpool.tile([128, 512], BF16, name="stsb",
                                     tag=f"stsb{h}{i % 3}")
                         for h in range(2)]
                for h in range(2):
                    for e in range(eh):
                        nc.tensor.transpose(
                            st_ps[h][:, e * 128:(e + 1) * 128],
                            Sv[:, e, h * 128:(h + 1) * 128],
                            ident_bf[:])
                    a.copy(st_sb[h][:, 0:W], st_ps[h][:, 0:W])
                # sims = A_hat . S_hat^T  (b on partitions, n on columns)
                p_ps = ps_p.tile([B, 512], F32, tag=f"p{i % 2}")
                nc.tensor.matmul(p_ps[:, 0:W], at_sb[:, 0, :],
                                 st_sb[0][:, 0:W], start=True, stop=False)
                nc.tensor.matmul(p_ps[:, 0:W], at_sb[:, 1, :],
                                 st_sb[1][:, 0:W], start=False, stop=True)
                e_scr = epool.tile([B, 512], F32, tag=f"e{i % 3}")
                a.activation(e_scr[:, 0:W], p_ps[:, 0:W], AF.Exp, scale=0.5,
                             accum_out=Eh[:, slot:slot + 1])

            # ---------- epilogue: partial scalars ----------
            E_sb = small.tile([B, K], F32)
            v.reduce_sum(E_sb[:, 0:3],
                         Eh[:, 0:6].rearrange("p (k g) -> p k g", g=2),
                         axis=mybir.AxisListType.X)
            v.reduce_sum(E_sb[:, 3:4],
                         Eh[:, 6:10].rearrange("p (k g) -> p k g", g=4),
                         axis=mybir.AxisListType.X)
            t4b = small.tile([B, K], F32)
            epb = small.tile([B, 1], F32)
            rsum = small.tile([B, 1], F32)
            enb = small.tile([B, 1], F32)
            v.scalar_tensor_tensor(t4b[:], E_sb[:], 1.0, oh4_sb[:],
                                   MUL, MUL, accum_out=epb[:])
            v.reduce_sum(rsum[:], E_sb[:], axis=mybir.AxisListType.X)
            v.tensor_scalar_mul(enb[:], epb[:], -1.0)
            v.tensor_add(enb[:], enb[:], rsum[:])
            F = small.tile([B, 4], F32)
            v.memset(F[:], 0.0)
            v.tensor_scalar_mul(F[:, 0:1], epb[:], 1.0 / (B * NB))
            v.tensor_scalar_mul(F[:, 1:2], enb[:], 1.0 / (B * (K - 1) * NB))
            v.tensor_copy(F[:, 2:3], contrib[:])
            out_ps = ps_p.tile([B, 512], F32, tag="p1")
            nc.tensor.matmul(out_ps[0:4, 0:1], F[:], ones_sb[0:B, :],
                             start=True, stop=True)
            out_sb = small.tile([4, 1], F32)
            a.copy(out_sb[:], out_ps[0:4, 0:1])
            nc.sync.dma_start(out=out_d[:], in_=out_sb[:])

    nc.compile()
    return nc


_NC = None


def _get_nc():
    global _NC
    if _NC is None:
        _NC = _build()
    return _NC


def _make_in_maps(hazards, S, indiv, gene, path, cohort_bank, label, c):
    hazards = np.asarray(hazards, dtype=np.float32)
    S = np.asarray(S, dtype=np.float32)
    indiv = np.asarray(indiv, dtype=np.float32)
    gene = np.asarray(gene, dtype=np.float32)
    path = np.asarray(path, dtype=np.float32)
    cohort_bank = np.asarray(cohort_bank, dtype=np.float32)
    label = np.asarray(label)
    c = np.asarray(c)

    oh5 = np.zeros((B, K + 1), np.float32)
    oh5[np.arange(B), label] = 1.0
    oh5b = np.zeros((B, K + 1), np.float32)
    oh5b[np.arange(B), label + 1] = 1.0
    sm = np.zeros((B, SM_W), np.float32)
    sm[:, 0:1024] = indiv.reshape(B, -1)
    sm[:, 1024:1280] = gene.reshape(B, -1)
    sm[:, 1280:1536] = path.reshape(B, -1)
    sm[:, 1536:1540] = hazards
    sm[:, 1540] = 1.0
    sm[:, 1541:1545] = S
    sm[:, 1545:1550] = oh5
    sm[:, 1550:1555] = oh5b
    sm[:, 1555:1559] = oh5[:, :K]
    sm[:, 1559] = c.astype(np.float32)
    sm[:, 1560] = 1.0 - c.astype(np.float32)
    sm = np.ascontiguousarray(sm)

    bankf = cohort_bank.reshape(K, NB, 1024)
    in_maps = []
    for i in range(NCORES):
        shard = np.ascontiguousarray(
            bankf[:, i * NSH:(i + 1) * NSH, :]).reshape(ROWS, 1024)
        in_maps.append({"smalls": sm, "bank": shard})
    return in_maps


_LAST_RESULTS = None  # stashed for test.py introspection


def kernel(hazards, S, indiv, gene, path, cohort_bank, label, c):
    global _LAST_RESULTS
    nc = _get_nc()
    in_maps = _make_in_maps(hazards, S, indiv, gene, path, cohort_bank, label, c)
    trace = bool(int(os.environ.get("TRNK_TRACE", "0")))
    res = run_bass_kernel_spmd(nc, in_maps, core_ids=list(range(NCORES)),
                               trace=trace)
    _LAST_RESULTS = res
    outs = np.stack([r["out_vec"][:, 0] for r in res.results])  # [8, 4]
    ep = float(outs[:, 0].sum())
    en = float(outs[:, 1].sum())
    other = float(outs[:, 2].mean())
    loss = other - math.log((ep + EPS_COH) / (ep + en + EPS_COH))
    return np.float32(loss)


# revision 34
# speedup vs baseline: 1.2215x; 1.2215x over previous
"""Trainium2 Bass kernel for nn_Loss_factory_12429635355015.

Loss = NLLSurv + CohortLoss(intra + inter) over a [4, 8192, 4, 256] cohort bank.

Strategy (memory-bound, 8 NeuronCores):
  - Shard cohort_bank along the N (bank-entry) axis: each core streams its
    16 MiB shard once at HBM line rate (4 tiles x 4 MiB contiguous SWDGE
    cast-DMAs, f32 HBM -> bf16 SBUF). One tile == one class (1024 entries).
  - ALL input DMAs go through the SWDGE queue, small tensors first: while
    the bank stream saturates the DMA engines, HWDGE-queue DMAs are starved
    for tens of us, so nothing else may ride the HWDGE queues mid-kernel.
  - Per 1024-entry tile, work is spread so DMA stays the bottleneck:
      DVE:  3 bf16 adds (component sum S) + broadcast normalize
      POOL: S*S and the per-entry sum-of-squares reduce (+ DMA desc-gens)
      ACT:  rsqrt via Ln+Exp on [128,8]; 2 PSUM->SBUF copies; exp(sims/tau)
            with accum_out giving per-class partial sums directly
      PE :  16 [128x128] transposes (S -> S^T) + 4 matmuls vs anchors
  - NLL + intra terms are computed on-device from host-encoded one-hots
    (index encoding only; all arithmetic on device).
  - Each core outputs [ep_partial, en_partial, nll+intra]; the host sums the
    two scalars across cores (the 'all-reduce two scalars' step) and applies
    the final -log((ep+eps)/(ep+en+eps)).
"""

import math
import os
import sys

import numpy as np

for _p in ("/opt/trn_rl_repo",):
    if _p not in sys.path and os.path.isdir(_p):
        sys.path.insert(0, _p)

import concourse.bacc as bacc
import concourse.tile as tile
from concourse import mybir
from concourse.bass_utils import run_bass_kernel_spmd

# Pin every activation to the one table set that contains all functions this
# kernel uses (Square/Ln/Exp/Copy/Abs/Identity). Without this, Bacc's
# first-match set selection alternates between sets (Ln lives outside the
# default exp set) and reloads the ACT tables ~1.3us per switch every tile.
_ACT_SET = "natural_log_exp_and_others"


def _pin_act_tables():
    import functools
    import concourse.hw_specs as hw_specs
    if getattr(hw_specs.get_activation_tables, "_pinned", False):
        return
    orig = hw_specs.get_activation_tables

    @functools.cache
    def pinned(arch):
        tabs = orig(arch)
        return {k: (v if k == _ACT_SET else set()) for k, v in tabs.items()}

    pinned._pinned = True
    hw_specs.get_activation_tables = pinned
    bacc.get_activation_tables = pinned


_pin_act_tables()

F32 = mybir.dt.float32
AF = mybir.ActivationFunctionType

# Problem constants (hardcoded per spec).
B = 64            # batch
K = 4             # n_cls
C = 256           # feature dim
NB = 8192         # bank entries per class (global)
NCORES = 8
NSH = NB // NCORES          # 1024 bank entries per class per core
ROWS = K * NSH              # 4096 rows of [4*256] per core
NT = 1024                   # bank entries per tile (4 MiB) == one class
TILES = ROWS // NT          # 4
EPG = NT // 128             # 8 entries per partition per tile
EPS_NLL = 1e-7
EPS_COH = 1e-8

# packed small-input layout: [B, SM_W] f32
#   indiv [0:1024) | gp [1024:1536) | haz [1536:1540) | spad [1540:1545)
#   ohy [1545:1550) | ohy1 [1550:1555) | oh4 [1555:1559) | cfs [1559:1561)
SM_W = 1568


def _build():
    nc = bacc.Bacc("TRN2", target_bir_lowering=False, debug=False,
                   enable_asserts=False, num_devices=NCORES)

    bank = nc.dram_tensor("bank", [ROWS, 1024], F32, kind="ExternalInput")
    smalls = nc.dram_tensor("smalls", [B, SM_W], F32, kind="ExternalInput")
    out_d = nc.dram_tensor("out_vec", [4, 1], F32, kind="ExternalOutput")

    import ml_dtypes
    ident_d = nc.inline_tensor(np.eye(128, dtype=np.float32), "ident")
    ident_bf_d = nc.inline_tensor(np.eye(128, dtype=ml_dtypes.bfloat16), "ident_bf")
    ones_d = nc.inline_tensor(np.ones((128, 1), dtype=np.float32), "ones_col")

    v = nc.vector
    a = nc.scalar
    g = nc.gpsimd

    with tile.TileContext(nc) as tc:
        from contextlib import ExitStack
        with ExitStack() as ctx:
            const = ctx.enter_context(tc.tile_pool(name="const", bufs=1))
            small = ctx.enter_context(tc.tile_pool(name="small", bufs=1))
            tpool = ctx.enter_context(tc.tile_pool(name="T", bufs=2))
            spool = ctx.enter_context(tc.tile_pool(name="S", bufs=2))
            stpool = ctx.enter_context(tc.tile_pool(name="STsb", bufs=2))
            epool = ctx.enter_context(tc.tile_pool(name="esb", bufs=2))
            ps_st = ctx.enter_context(tc.tile_pool(name="ps_st", bufs=1, space="PSUM"))
            ps_p = ctx.enter_context(tc.tile_pool(name="ps_p", bufs=1, space="PSUM"))
            ps_one = ctx.enter_context(tc.tile_pool(name="ps_one", bufs=1, space="PSUM"))

            BF16 = mybir.dt.bfloat16

            # ---------- all input DMAs on the SWDGE queue, smalls first ----
            sm_sb = small.tile([B, SM_W], F32)
            g.dma_start(out=sm_sb[:], in_=smalls[:])
            ident_sb = const.tile([128, 128], F32)
            g.dma_start(out=ident_sb[:], in_=ident_d[:])
            ident_bf = const.tile([128, 128], BF16)
            g.dma_start(out=ident_bf[:], in_=ident_bf_d[:])
            ones_sb = const.tile([128, 1], F32)
            g.dma_start(out=ones_sb[:], in_=ones_d[:])

            # Chunked streaming: tiles 0-2 as 2 MiB halves, tile 3 (the last
            # class) as 1 MiB quarters so the drain chain after the final DMA
            # byte is as short as possible.  chunk = (tile, e-offset, e-count,
            # accum slot).
            CHUNKS = [(t, q * 4, 4, 2 * t + q)
                      for t in range(3) for q in range(2)]
            CHUNKS += [(3, q * 2, 2, 6 + q) for q in range(4)]
            NCH = len(CHUNKS)
            NSLOT = 10

            T_sb = {}
            for i in range(NCH):
                T_sb[i] = tpool.tile([128, 4 * 1024], BF16,
                                     name="Tsb", tag=f"T{i % 2}")

            def emit_bank_dma(i):
                t, e0, eh, _ = CHUNKS[i]
                src = bank[t * NT:(t + 1) * NT, :].rearrange(
                    "(p e) x -> p e x", e=EPG)[:, e0:e0 + eh, :]
                g.dma_start(
                    out=T_sb[i][:, 0:eh * 1024].rearrange(
                        "p (e x) -> p e x", e=eh),
                    in_=src)

            for i in range(4):
                emit_bank_dma(i)

            # views into the packed small-input tile
            ind_sb = sm_sb[:, 0:1024]
            gp_sb = sm_sb[:, 1024:1536]
            haz_sb = sm_sb[:, 1536:1540]
            spad_sb = sm_sb[:, 1540:1545]
            ohy_sb = sm_sb[:, 1545:1550]
            ohy1_sb = sm_sb[:, 1550:1555]
            oh4_sb = sm_sb[:, 1555:1559]
            cfs_sb = sm_sb[:, 1559:1561]

            # ---------- anchors: A = l2norm(mean_j indiv[b,j,:]) ----------
            iv = ind_sb.rearrange("p (j c) -> p j c", j=4)
            asum = small.tile([B, C], F32)
            atmp = small.tile([B, C], F32)
            v.tensor_add(asum[:], iv[:, 0, :], iv[:, 1, :])
            v.tensor_add(atmp[:], iv[:, 2, :], iv[:, 3, :])
            v.tensor_add(asum[:], asum[:], atmp[:])
            sqa = small.tile([B, C], F32)
            ssa = small.tile([B, 1], F32)
            a.activation(sqa[:], asum[:], AF.Square, accum_out=ssa[:])
            lna = small.tile([B, 1], F32)
            a.activation(lna[:], ssa[:], AF.Ln)
            rsa = small.tile([B, 1], F32)
            a.activation(rsa[:], lna[:], AF.Exp, scale=-0.5)
            v.tensor_scalar_mul(asum[:], asum[:], rsa[:])
            at_ps = ps_one.tile([128, 2, B], F32, tag="at")
            for h in range(2):
                nc.tensor.transpose(at_ps[:, h, :], asum[:, h * 128:(h + 1) * 128],
                                    ident_sb[0:B, 0:B])
            at_sb = const.tile([128, 2, B], BF16)
            a.copy(at_sb[:], at_ps[:])

            # ---------- NLL (per-b, b on partitions) ----------
            MUL = mybir.AluOpType.mult
            t5 = small.tile([B, K + 1], F32)
            t4 = small.tile([B, K], F32)
            sy = small.tile([B, 1], F32)
            hy = small.tile([B, 1], F32)
            sy1 = small.tile([B, 1], F32)
            v.scalar_tensor_tensor(t5[:], spad_sb[:], 1.0, ohy_sb[:],
                                   MUL, MUL, accum_out=sy[:])
            v.scalar_tensor_tensor(t4[:], haz_sb[:], 1.0, ohy_sb[:, 0:K],
                                   MUL, MUL, accum_out=hy[:])
            v.scalar_tensor_tensor(t5[:], spad_sb[:], 1.0, ohy1_sb[:],
                                   MUL, MUL, accum_out=sy1[:])
            for x in (sy, hy, sy1):
                v.tensor_scalar_max(x[:], x[:], EPS_NLL)
            lsy = small.tile([B, 1], F32)
            lhy = small.tile([B, 1], F32)
            lsy1 = small.tile([B, 1], F32)
            a.activation(lsy[:], sy[:], AF.Ln)
            a.activation(lhy[:], hy[:], AF.Ln)
            a.activation(lsy1[:], sy1[:], AF.Ln)
            tu = small.tile([B, 1], F32)
            tcen = small.tile([B, 1], F32)
            negl = small.tile([B, 1], F32)
            v.tensor_add(tu[:], lsy[:], lhy[:])
            v.tensor_mul(tu[:], tu[:], cfs_sb[:, 1:2])      # *(1-cf)
            v.tensor_mul(tcen[:], lsy1[:], cfs_sb[:, 0:1])  # *cf
            v.tensor_add(negl[:], tu[:], tcen[:])           # = -neg_l per b

            # ---------- intra cohort term ----------
            # dots on raw vectors, then rescale D by rsi[p]*rsg[t] afterward
            prod = small.tile([B, C], F32)
            ssqi = small.tile([B, 4], F32)
            for j in range(4):
                v.scalar_tensor_tensor(prod[:], iv[:, j, :], 1.0, iv[:, j, :],
                                       MUL, MUL, accum_out=ssqi[:, j:j + 1])
            rsi = small.tile([B, 4], F32)
            a.activation(rsi[:], ssqi[:], AF.Ln)
            a.activation(rsi[:], rsi[:], AF.Exp, scale=-0.5)
            gv = gp_sb.rearrange("p (t c) -> p t c", t=2)
            ssqg = small.tile([B, 2], F32)
            for tt in range(2):
                v.scalar_tensor_tensor(prod[:], gv[:, tt, :], 1.0, gv[:, tt, :],
                                       MUL, MUL, accum_out=ssqg[:, tt:tt + 1])
            rsg = small.tile([B, 2], F32)
            a.activation(rsg[:], ssqg[:], AF.Ln)
            a.activation(rsg[:], rsg[:], AF.Exp, scale=-0.5)
            D = small.tile([B, 8], F32)
            for p in range(4):
                for t in range(2):
                    col = p * 2 + t
                    v.scalar_tensor_tensor(prod[:], iv[:, p, :], 1.0,
                                           gv[:, t, :], MUL, MUL,
                                           accum_out=D[:, col:col + 1])
            Dv = D.rearrange("p (j t) -> p j t", t=2)
            v.tensor_mul(Dv[:], Dv[:], rsi[:, :, None].broadcast_to([B, 4, 2]))
            v.tensor_mul(Dv[:], Dv[:], rsg[:, None, :].broadcast_to([B, 4, 2]))
            U = small.tile([B, 8], F32)
            a.activation(U[:], D[:], AF.Abs)
            # mask==1 entries (cols 0,1,4,7) use -sim instead of |sim|
            v.tensor_scalar_mul(U[:, 0:2], D[:, 0:2], -1.0)
            v.tensor_scalar_mul(U[:, 4:5], D[:, 4:5], -1.0)
            v.tensor_scalar_mul(U[:, 7:8], D[:, 7:8], -1.0)
            isum = small.tile([B, 1], F32)
            v.reduce_sum(isum[:], U[:], axis=mybir.AxisListType.X)
            # contrib_b = -negl/B + isum/(8B) + 1/B  -> sums to nll + intra_loss
            c1 = small.tile([B, 1], F32)
            c2 = small.tile([B, 1], F32)
            contrib = small.tile([B, 1], F32)
            v.tensor_scalar_mul(c1[:], negl[:], -1.0 / B)
            v.tensor_scalar_mul(c2[:], isum[:], 1.0 / (8 * B))
            v.tensor_add(contrib[:], c1[:], c2[:])
            v.tensor_scalar_add(contrib[:], contrib[:], 1.0 / B)

            # ---------- main loop over bank chunks ----------
            Eh = small.tile([B, NSLOT], F32)   # accum_out per chunk
            for i in range(NCH):
                t, e0, eh, slot = CHUNKS[i]
                if i + 4 < NCH:
                    emit_bank_dma(i + 4)
                W = eh * 128   # sims columns in this chunk
                Tv = T_sb[i][:, 0:eh * 1024].rearrange(
                    "p (e j c) -> p e j c", e=eh, j=4)
                S_sb = spool.tile([128, 4 * C], BF16, tag=f"S{i % 2}")
                Sv = S_sb[:, 0:eh * C].rearrange("p (e c) -> p e c", e=eh)
                tmp = spool.tile([128, 4 * C], BF16, tag=f"tmp{i % 2}")
                tv = tmp[:, 0:eh * C].rearrange("p (e c) -> p e c", e=eh)
                v.tensor_add(Sv[:], Tv[:, :, 0, :], Tv[:, :, 1, :])
                v.tensor_add(tv[:], Tv[:, :, 2, :], Tv[:, :, 3, :])
                v.tensor_add(Sv[:], Sv[:], tv[:])
                # S*S on the pool engine while DMA streams; on DVE during the
                # post-stream drain (shorter chain)
                sq = spool.tile([128, 4 * C], BF16, tag=f"sq{i % 2}")
                sqv = sq[:, 0:eh * C]
                (v if i >= 8 else g).tensor_mul(sqv[:], S_sb[:, 0:eh * C],
                                                S_sb[:, 0:eh * C])
                ss = spool.tile([128, 4], F32, tag=f"ss{i % 2}")
                v.reduce_sum(ss[:, 0:eh],
                             sqv.rearrange("p (e c) -> p e c", e=eh),
                             axis=mybir.AxisListType.X)
                rr = spool.tile([128, 4], F32, tag=f"rr{i % 2}")
                a.activation(rr[:, 0:eh], ss[:, 0:eh], AF.Ln)
                a.activation(rr[:, 0:eh], rr[:, 0:eh], AF.Exp, scale=-0.5)
                v.tensor_mul(Sv[:], Sv[:],
                             rr[:, 0:eh, None].broadcast_to([128, eh, C]))
                # transpose normalized S into [c, n] chunks (h = c-half)
                st_ps = [ps_st.tile([128, 512], BF16, name="stps",
                                    tag=f"stps{h}{i % 2}")
                         for h in range(2)]
                st_sb = [stpool.tile([128, 512], BF16, name="stsb",
                                     tag=f"stsb{h}{i % 2}")
                         for h in range(2)]
                for h in range(2):
                    for e in range(eh):
                        nc.tensor.transpose(
                            st_ps[h][:, e * 128:(e + 1) * 128],
                            Sv[:, e, h * 128:(h + 1) * 128],
                            ident_bf[:])
                    a.copy(st_sb[h][:, 0:W], st_ps[h][:, 0:W])
                # sims = A_hat . S_hat^T  (b on partitions, n on columns)
                p_ps = ps_p.tile([B, 512], F32, tag=f"p{i % 2}")
                nc.tensor.matmul(p_ps[:, 0:W], at_sb[:, 0, :],
                                 st_sb[0][:, 0:W], start=True, stop=False)
                nc.tensor.matmul(p_ps[:, 0:W], at_sb[:, 1, :],
                                 st_sb[1][:, 0:W], start=False, stop=True)
                e_scr = epool.tile([B, 512], F32, tag=f"e{i % 2}")
                a.activation(e_scr[:, 0:W], p_ps[:, 0:W], AF.Exp, scale=0.5,
                             accum_out=Eh[:, slot:slot + 1])

            # ---------- epilogue: partial scalars ----------
            E_sb = small.tile([B, K], F32)
            v.reduce_sum(E_sb[:, 0:3],
                         Eh[:, 0:6].rearrange("p (k g) -> p k g", g=2),
                         axis=mybir.AxisListType.X)
            v.reduce_sum(E_sb[:, 3:4],
                         Eh[:, 6:10].rearrange("p (k g) -> p k g", g=4),
                         axis=mybir.AxisListType.X)
            t4b = small.tile([B, K], F32)
            epb = small.tile([B, 1], F32)
            rsum = small.tile([B, 1], F32)
            enb = small.tile([B, 1], F32)
            v.scalar_tensor_tensor(t4b[:], E_sb[:], 1.0, oh4_sb[:],
                                   MUL, MUL, accum_out=epb[:])
            v.reduce_sum(rsum[:], E_sb[:], axis=mybir.AxisListType.X)
            v.tensor_scalar_mul(enb[:], epb[:], -1.0)
            v.tensor_add(enb[:], enb[:], rsum[:])
            F = small.tile([B, 4], F32)
            v.memset(F[:], 0.0)
            v.tensor_scalar_mul(F[:, 0:1], epb[:], 1.0 / (B * NB))
            v.tensor_scalar_mul(F[:, 1:2], enb[:], 1.0 / (B * (K - 1) * NB))
            v.tensor_copy(F[:, 2:3], contrib[:])
            out_ps = ps_one.tile([4, 1], F32, tag="o3")
            nc.tensor.matmul(out_ps[:], F[:], ones_sb[0:B, :], start=True, stop=True)
            out_sb = small.tile([4, 1], F32)
            a.copy(out_sb[:], out_ps[:])
            nc.sync.dma_start(out=out_d[:], in_=out_sb[:])

    nc.compile()
    return nc


_NC = None


def _get_nc():
    global _NC
    if _NC is None:
        _NC = _build()
    return _NC


def _make_in_maps(hazards, S, indiv, gene, path, cohort_bank, label, c):
    hazards = np.asarray(hazards, dtype=np.float32)
    S = np.asarray(S, dtype=np.float32)
    indiv = np.asarray(indiv, dtype=np.float32)
    gene = np.asarray(gene, dtype=np.float32)
    path = np.asarray(path, dtype=np.float32)
    cohort_bank = np.asarray(cohort_bank, dtype=np.float32)
    label = np.asarray(label)
    c = np.asarray(c)

    oh5 = np.zeros((B, K + 1), np.float32)
    oh5[np.arange(B), label] = 1.0
    oh5b = np.zeros((B, K + 1), np.float32)
    oh5b[np.arange(B), label + 1] = 1.0
    sm = np.zeros((B, SM_W), np.float32)
    sm[:, 0:1024] = indiv.reshape(B, -1)
    sm[:, 1024:1280] = gene.reshape(B, -1)
    sm[:, 1280:1536] = path.reshape(B, -1)
    sm[:, 1536:1540] = hazards
    sm[:, 1540] = 1.0
    sm[:, 1541:1545] = S
    sm[:, 1545:1550] = oh5
    sm[:, 1550:1555] = oh5b
    sm[:, 1555:1559] = oh5[:, :K]
    sm[:, 1559] = c.astype(np.float32)
    sm[:, 1560] = 1.0 - c.astype(np.float32)
    sm = np.ascontiguousarray(sm)

    bankf = cohort_bank.reshape(K, NB, 1024)
    in_maps = []
    for i in range(NCORES):
        shard = np.ascontiguousarray(
            bankf[:, i * NSH:(i + 1) * NSH, :]).reshape(ROWS, 1024)
        in_maps.append({"smalls": sm, "bank": shard})
    return in_maps


_LAST_RESULTS = None  # stashed for test.py introspection


def kernel(hazards, S, indiv, gene, path, cohort_bank, label, c):
    global _LAST_RESULTS
    nc = _get_nc()
    in_maps = _make_in_maps(hazards, S, indiv, gene, path, cohort_bank, label, c)
    trace = bool(int(os.environ.get("TRNK_TRACE", "0")))
    res = run_bass_kernel_spmd(nc, in_maps, core_ids=list(range(NCORES)),
                               trace=trace)
    _LAST_RESULTS = res
    outs = np.stack([r["out_vec"][:, 0] for r in res.results])  # [8, 4]
    ep = float(outs[:, 0].sum())
    en = float(outs[:, 1].sum())
    other = float(outs[:, 2].mean())
    loss = other - math.log((ep + EPS_COH) / (ep + en + EPS_COH))
    return np.float32(loss)


# revision 35
# speedup vs baseline: 1.2236x; 1.0017x over previous
"""Trainium2 Bass kernel for nn_Loss_factory_12429635355015.

Loss = NLLSurv + CohortLoss(intra + inter) over a [4, 8192, 4, 256] cohort bank.

Strategy (memory-bound, 8 NeuronCores):
  - Shard cohort_bank along the N (bank-entry) axis: each core streams its
    16 MiB shard once at HBM line rate (4 tiles x 4 MiB contiguous SWDGE
    cast-DMAs, f32 HBM -> bf16 SBUF). One tile == one class (1024 entries).
  - ALL input DMAs go through the SWDGE queue, small tensors first: while
    the bank stream saturates the DMA engines, HWDGE-queue DMAs are starved
    for tens of us, so nothing else may ride the HWDGE queues mid-kernel.
  - Per 1024-entry tile, work is spread so DMA stays the bottleneck:
      DVE:  3 bf16 adds (component sum S) + broadcast normalize
      POOL: S*S and the per-entry sum-of-squares reduce (+ DMA desc-gens)
      ACT:  rsqrt via Ln+Exp on [128,8]; 2 PSUM->SBUF copies; exp(sims/tau)
            with accum_out giving per-class partial sums directly
      PE :  16 [128x128] transposes (S -> S^T) + 4 matmuls vs anchors
  - NLL + intra terms are computed on-device from host-encoded one-hots
    (index encoding only; all arithmetic on device).
  - Each core outputs [ep_partial, en_partial, nll+intra]; the host sums the
    two scalars across cores (the 'all-reduce two scalars' step) and applies
    the final -log((ep+eps)/(ep+en+eps)).
"""

import math
import os
import sys

import numpy as np

for _p in ("/opt/trn_rl_repo",):
    if _p not in sys.path and os.path.isdir(_p):
        sys.path.insert(0, _p)

import concourse.bacc as bacc
import concourse.tile as tile
from concourse import mybir
from concourse.bass_utils import run_bass_kernel_spmd

# Pin every activation to the one table set that contains all functions this
# kernel uses (Square/Ln/Exp/Copy/Abs/Identity). Without this, Bacc's
# first-match set selection alternates between sets (Ln lives outside the
# default exp set) and reloads the ACT tables ~1.3us per switch every tile.
_ACT_SET = "natural_log_exp_and_others"


def _pin_act_tables():
    import functools
    import concourse.hw_specs as hw_specs
    if getattr(hw_specs.get_activation_tables, "_pinned", False):
        return
    orig = hw_specs.get_activation_tables

    @functools.cache
    def pinned(arch):
        tabs = orig(arch)
        return {k: (v if k == _ACT_SET else set()) for k, v in tabs.items()}

    pinned._pinned = True
    hw_specs.get_activation_tables = pinned
    bacc.get_activation_tables = pinned


_pin_act_tables()

F32 = mybir.dt.float32
AF = mybir.ActivationFunctionType

# Problem constants (hardcoded per spec).
B = 64            # batch
K = 4             # n_cls
C = 256           # feature dim
NB = 8192         # bank entries per class (global)
NCORES = 8
NSH = NB // NCORES          # 1024 bank entries per class per core
ROWS = K * NSH              # 4096 rows of [4*256] per core
NT = 1024                   # bank entries per tile (4 MiB) == one class
TILES = ROWS // NT          # 4
EPG = NT // 128             # 8 entries per partition per tile
EPS_NLL = 1e-7
EPS_COH = 1e-8

# packed small-input layout: [B, SM_W] f32
#   indiv [0:1024) | gp [1024:1536) | haz [1536:1540) | spad [1540:1545)
#   ohy [1545:1550) | ohy1 [1550:1555) | oh4 [1555:1559) | cfs [1559:1561)
SM_W = 1568


def _build():
    nc = bacc.Bacc("TRN2", target_bir_lowering=False, debug=False,
                   enable_asserts=False, num_devices=NCORES)

    bank = nc.dram_tensor("bank", [ROWS, 1024], F32, kind="ExternalInput")
    smalls = nc.dram_tensor("smalls", [B, SM_W], F32, kind="ExternalInput")
    out_d = nc.dram_tensor("out_vec", [4, 1], F32, kind="ExternalOutput")

    import ml_dtypes
    ident_d = nc.inline_tensor(np.eye(128, dtype=np.float32), "ident")
    ident_bf_d = nc.inline_tensor(np.eye(128, dtype=ml_dtypes.bfloat16), "ident_bf")
    ones_d = nc.inline_tensor(np.ones((128, 1), dtype=np.float32), "ones_col")

    v = nc.vector
    a = nc.scalar
    g = nc.gpsimd

    with tile.TileContext(nc) as tc:
        from contextlib import ExitStack
        with ExitStack() as ctx:
            const = ctx.enter_context(tc.tile_pool(name="const", bufs=1))
            small = ctx.enter_context(tc.tile_pool(name="small", bufs=1))
            tpool = ctx.enter_context(tc.tile_pool(name="T", bufs=2))
            spool = ctx.enter_context(tc.tile_pool(name="S", bufs=2))
            stpool = ctx.enter_context(tc.tile_pool(name="STsb", bufs=2))
            epool = ctx.enter_context(tc.tile_pool(name="esb", bufs=2))
            ps_st = ctx.enter_context(tc.tile_pool(name="ps_st", bufs=1, space="PSUM"))
            ps_p = ctx.enter_context(tc.tile_pool(name="ps_p", bufs=1, space="PSUM"))
            ps_one = ctx.enter_context(tc.tile_pool(name="ps_one", bufs=1, space="PSUM"))

            BF16 = mybir.dt.bfloat16

            # ---------- all input DMAs on the SWDGE queue, smalls first ----
            sm_sb = small.tile([B, SM_W], F32)
            g.dma_start(out=sm_sb[:], in_=smalls[:])
            ident_sb = const.tile([128, 128], F32)
            g.dma_start(out=ident_sb[:], in_=ident_d[:])
            ident_bf = const.tile([128, 128], BF16)
            g.dma_start(out=ident_bf[:], in_=ident_bf_d[:])
            ones_sb = const.tile([128, 1], F32)
            g.dma_start(out=ones_sb[:], in_=ones_d[:])

            # Chunked streaming: tiles 0-2 as 2 MiB halves, tile 3 (the last
            # class) as 1 MiB quarters so the drain chain after the final DMA
            # byte is as short as possible.  chunk = (tile, e-offset, e-count,
            # accum slot).
            CHUNKS = [(t, q * 4, 4, 2 * t + q)
                      for t in range(3) for q in range(2)]
            CHUNKS += [(3, q * 2, 2, 6 + q) for q in range(4)]
            NCH = len(CHUNKS)
            NSLOT = 10

            T_sb = {}
            for i in range(NCH):
                T_sb[i] = tpool.tile([128, 4 * 1024], BF16,
                                     name="Tsb", tag=f"T{i % 2}")

            def emit_bank_dma(i):
                t, e0, eh, _ = CHUNKS[i]
                src = bank[t * NT:(t + 1) * NT, :].rearrange(
                    "(p e) x -> p e x", e=EPG)[:, e0:e0 + eh, :]
                g.dma_start(
                    out=T_sb[i][:, 0:eh * 1024].rearrange(
                        "p (e x) -> p e x", e=eh),
                    in_=src)

            for i in range(4):
                emit_bank_dma(i)

            # views into the packed small-input tile
            ind_sb = sm_sb[:, 0:1024]
            gp_sb = sm_sb[:, 1024:1536]
            haz_sb = sm_sb[:, 1536:1540]
            spad_sb = sm_sb[:, 1540:1545]
            ohy_sb = sm_sb[:, 1545:1550]
            ohy1_sb = sm_sb[:, 1550:1555]
            oh4_sb = sm_sb[:, 1555:1559]
            cfs_sb = sm_sb[:, 1559:1561]

            # ---------- anchors: A = l2norm(mean_j indiv[b,j,:]) ----------
            iv = ind_sb.rearrange("p (j c) -> p j c", j=4)
            asum = small.tile([B, C], F32)
            atmp = small.tile([B, C], F32)
            v.tensor_add(asum[:], iv[:, 0, :], iv[:, 1, :])
            v.tensor_add(atmp[:], iv[:, 2, :], iv[:, 3, :])
            v.tensor_add(asum[:], asum[:], atmp[:])
            sqa = small.tile([B, C], F32)
            ssa = small.tile([B, 1], F32)
            a.activation(sqa[:], asum[:], AF.Square, accum_out=ssa[:])
            lna = small.tile([B, 1], F32)
            a.activation(lna[:], ssa[:], AF.Ln)
            rsa = small.tile([B, 1], F32)
            a.activation(rsa[:], lna[:], AF.Exp, scale=-0.5)
            v.tensor_scalar_mul(asum[:], asum[:], rsa[:])
            at_ps = ps_one.tile([128, 2, B], F32, tag="at")
            for h in range(2):
                nc.tensor.transpose(at_ps[:, h, :], asum[:, h * 128:(h + 1) * 128],
                                    ident_sb[0:B, 0:B])
            at_sb = const.tile([128, 2, B], BF16)
            a.copy(at_sb[:], at_ps[:])

            # ---------- NLL (per-b, b on partitions) ----------
            MUL = mybir.AluOpType.mult
            t5 = small.tile([B, K + 1], F32)
            t4 = small.tile([B, K], F32)
            sy = small.tile([B, 1], F32)
            hy = small.tile([B, 1], F32)
            sy1 = small.tile([B, 1], F32)
            v.scalar_tensor_tensor(t5[:], spad_sb[:], 1.0, ohy_sb[:],
                                   MUL, MUL, accum_out=sy[:])
            v.scalar_tensor_tensor(t4[:], haz_sb[:], 1.0, ohy_sb[:, 0:K],
                                   MUL, MUL, accum_out=hy[:])
            v.scalar_tensor_tensor(t5[:], spad_sb[:], 1.0, ohy1_sb[:],
                                   MUL, MUL, accum_out=sy1[:])
            for x in (sy, hy, sy1):
                v.tensor_scalar_max(x[:], x[:], EPS_NLL)
            lsy = small.tile([B, 1], F32)
            lhy = small.tile([B, 1], F32)
            lsy1 = small.tile([B, 1], F32)
            a.activation(lsy[:], sy[:], AF.Ln)
            a.activation(lhy[:], hy[:], AF.Ln)
            a.activation(lsy1[:], sy1[:], AF.Ln)
            tu = small.tile([B, 1], F32)
            tcen = small.tile([B, 1], F32)
            negl = small.tile([B, 1], F32)
            v.tensor_add(tu[:], lsy[:], lhy[:])
            v.tensor_mul(tu[:], tu[:], cfs_sb[:, 1:2])      # *(1-cf)
            v.tensor_mul(tcen[:], lsy1[:], cfs_sb[:, 0:1])  # *cf
            v.tensor_add(negl[:], tu[:], tcen[:])           # = -neg_l per b

            # ---------- intra cohort term ----------
            # dots on raw vectors, then rescale D by rsi[p]*rsg[t] afterward
            prod = small.tile([B, C], F32)
            ssqi = small.tile([B, 4], F32)
            for j in range(4):
                a.activation(sqa[:], iv[:, j, :], AF.Square,
                             accum_out=ssqi[:, j:j + 1])
            rsi = small.tile([B, 4], F32)
            a.activation(rsi[:], ssqi[:], AF.Ln)
            a.activation(rsi[:], rsi[:], AF.Exp, scale=-0.5)
            gv = gp_sb.rearrange("p (t c) -> p t c", t=2)
            ssqg = small.tile([B, 2], F32)
            for tt in range(2):
                a.activation(sqa[:], gv[:, tt, :], AF.Square,
                             accum_out=ssqg[:, tt:tt + 1])
            rsg = small.tile([B, 2], F32)
            a.activation(rsg[:], ssqg[:], AF.Ln)
            a.activation(rsg[:], rsg[:], AF.Exp, scale=-0.5)
            D = small.tile([B, 8], F32)
            for p in range(4):
                for t in range(2):
                    col = p * 2 + t
                    v.scalar_tensor_tensor(prod[:], iv[:, p, :], 1.0,
                                           gv[:, t, :], MUL, MUL,
                                           accum_out=D[:, col:col + 1])
            Dv = D.rearrange("p (j t) -> p j t", t=2)
            v.tensor_mul(Dv[:], Dv[:], rsi[:, :, None].broadcast_to([B, 4, 2]))
            v.tensor_mul(Dv[:], Dv[:], rsg[:, None, :].broadcast_to([B, 4, 2]))
            U = small.tile([B, 8], F32)
            a.activation(U[:], D[:], AF.Abs)
            # mask==1 entries (cols 0,1,4,7) use -sim instead of |sim|
            v.tensor_scalar_mul(U[:, 0:2], D[:, 0:2], -1.0)
            v.tensor_scalar_mul(U[:, 4:5], D[:, 4:5], -1.0)
            v.tensor_scalar_mul(U[:, 7:8], D[:, 7:8], -1.0)
            isum = small.tile([B, 1], F32)
            v.reduce_sum(isum[:], U[:], axis=mybir.AxisListType.X)
            # contrib_b = -negl/B + isum/(8B) + 1/B  -> sums to nll + intra_loss
            c1 = small.tile([B, 1], F32)
            c2 = small.tile([B, 1], F32)
            contrib = small.tile([B, 1], F32)
            v.tensor_scalar_mul(c1[:], negl[:], -1.0 / B)
            v.tensor_scalar_mul(c2[:], isum[:], 1.0 / (8 * B))
            v.tensor_add(contrib[:], c1[:], c2[:])
            v.tensor_scalar_add(contrib[:], contrib[:], 1.0 / B)

            # ---------- main loop over bank chunks ----------
            Eh = small.tile([B, NSLOT], F32)   # accum_out per chunk
            for i in range(NCH):
                t, e0, eh, slot = CHUNKS[i]
                if i + 4 < NCH:
                    emit_bank_dma(i + 4)
                W = eh * 128   # sims columns in this chunk
                Tv = T_sb[i][:, 0:eh * 1024].rearrange(
                    "p (e j c) -> p e j c", e=eh, j=4)
                S_sb = spool.tile([128, 4 * C], BF16, tag=f"S{i % 2}")
                Sv = S_sb[:, 0:eh * C].rearrange("p (e c) -> p e c", e=eh)
                tmp = spool.tile([128, 4 * C], BF16, tag=f"tmp{i % 2}")
                tv = tmp[:, 0:eh * C].rearrange("p (e c) -> p e c", e=eh)
                v.tensor_add(Sv[:], Tv[:, :, 0, :], Tv[:, :, 1, :])
                v.tensor_add(tv[:], Tv[:, :, 2, :], Tv[:, :, 3, :])
                v.tensor_add(Sv[:], Sv[:], tv[:])
                # S*S on the pool engine while DMA streams; on DVE during the
                # post-stream drain (shorter chain)
                sq = spool.tile([128, 4 * C], BF16, tag=f"sq{i % 2}")
                sqv = sq[:, 0:eh * C]
                (v if i >= 8 else g).tensor_mul(sqv[:], S_sb[:, 0:eh * C],
                                                S_sb[:, 0:eh * C])
                ss = spool.tile([128, 4], F32, tag=f"ss{i % 2}")
                v.reduce_sum(ss[:, 0:eh],
                             sqv.rearrange("p (e c) -> p e c", e=eh),
                             axis=mybir.AxisListType.X)
                rr = spool.tile([128, 4], F32, tag=f"rr{i % 2}")
                a.activation(rr[:, 0:eh], ss[:, 0:eh], AF.Ln)
                a.activation(rr[:, 0:eh], rr[:, 0:eh], AF.Exp, scale=-0.5)
                v.tensor_mul(Sv[:], Sv[:],
                             rr[:, 0:eh, None].broadcast_to([128, eh, C]))
                # transpose normalized S into [c, n] chunks (h = c-half)
                st_ps = [ps_st.tile([128, 512], BF16, name="stps",
                                    tag=f"stps{h}{i % 2}")
                         for h in range(2)]
                st_sb = [stpool.tile([128, 512], BF16, name="stsb",
                                     tag=f"stsb{h}{i % 2}")
                         for h in range(2)]
                for h in range(2):
                    for e in range(eh):
                        nc.tensor.transpose(
                            st_ps[h][:, e * 128:(e + 1) * 128],
                            Sv[:, e, h * 128:(h + 1) * 128],
                            ident_bf[:])
                    a.copy(st_sb[h][:, 0:W], st_ps[h][:, 0:W])
                # sims = A_hat . S_hat^T  (b on partitions, n on columns)
                p_ps = ps_p.tile([B, 512], F32, tag=f"p{i % 2}")
                nc.tensor.matmul(p_ps[:, 0:W], at_sb[:, 0, :],
                                 st_sb[0][:, 0:W], start=True, stop=False)
                nc.tensor.matmul(p_ps[:, 0:W], at_sb[:, 1, :],
                                 st_sb[1][:, 0:W], start=False, stop=True)
                e_scr = epool.tile([B, 512], F32, tag=f"e{i % 2}")
                a.activation(e_scr[:, 0:W], p_ps[:, 0:W], AF.Exp, scale=0.5,
                             accum_out=Eh[:, slot:slot + 1])

            # ---------- epilogue: partial scalars ----------
            E_sb = small.tile([B, K], F32)
            v.reduce_sum(E_sb[:, 0:3],
                         Eh[:, 0:6].rearrange("p (k g) -> p k g", g=2),
                         axis=mybir.AxisListType.X)
            v.reduce_sum(E_sb[:, 3:4],
                         Eh[:, 6:10].rearrange("p (k g) -> p k g", g=4),
                         axis=mybir.AxisListType.X)
            t4b = small.tile([B, K], F32)
            epb = small.tile([B, 1], F32)
            rsum = small.tile([B, 1], F32)
            enb = small.tile([B, 1], F32)
            v.scalar_tensor_tensor(t4b[:], E_sb[:], 1.0, oh4_sb[:],
                                   MUL, MUL, accum_out=epb[:])
            v.reduce_sum(rsum[:], E_sb[:], axis=mybir.AxisListType.X)
            v.tensor_scalar_mul(enb[:], epb[:], -1.0)
            v.tensor_add(enb[:], enb[:], rsum[:])
            F = small.tile([B, 4], F32)
            v.memset(F[:], 0.0)
            v.tensor_scalar_mul(F[:, 0:1], epb[:], 1.0 / (B * NB))
            v.tensor_scalar_mul(F[:, 1:2], enb[:], 1.0 / (B * (K - 1) * NB))
            v.tensor_copy(F[:, 2:3], contrib[:])
            out_ps = ps_one.tile([4, 1], F32, tag="o3")
            nc.tensor.matmul(out_ps[:], F[:], ones_sb[0:B, :], start=True, stop=True)
            out_sb = small.tile([4, 1], F32)
            a.copy(out_sb[:], out_ps[:])
            nc.sync.dma_start(out=out_d[:], in_=out_sb[:])

    nc.compile()
    return nc


_NC = None


def _get_nc():
    global _NC
    if _NC is None:
        _NC = _build()
    return _NC


def _make_in_maps(hazards, S, indiv, gene, path, cohort_bank, label, c):
    hazards = np.asarray(hazards, dtype=np.float32)
    S = np.asarray(S, dtype=np.float32)
    indiv = np.asarray(indiv, dtype=np.float32)
    gene = np.asarray(gene, dtype=np.float32)
    path = np.asarray(path, dtype=np.float32)
    cohort_bank = np.asarray(cohort_bank, dtype=np.float32)
    label = np.asarray(label)
    c = np.asarray(c)

    oh5 = np.zeros((B, K + 1), np.float32)
    oh5[np.arange(B), label] = 1.0
    oh5b = np.zeros((B, K + 1), np.float32)
    oh5b[np.arange(B), label + 1] = 1.0
    sm = np.zeros((B, SM_W), np.float32)
    sm[:, 0:1024] = indiv.reshape(B, -1)
    sm[:, 1024:1280] = gene.reshape(B, -1)
    sm[:, 1280:1536] = path.reshape(B, -1)
    sm[:, 1536:1540] = hazards
    sm[:, 1540] = 1.0
    sm[:, 1541:1545] = S
    sm[:, 1545:1550] = oh5
    sm[:, 1550:1555] = oh5b
    sm[:, 1555:1559] = oh5[:, :K]
    sm[:, 1559] = c.astype(np.float32)
    sm[:, 1560] = 1.0 - c.astype(np.float32)
    sm = np.ascontiguousarray(sm)

    bankf = cohort_bank.reshape(K, NB, 1024)
    in_maps = []
    for i in range(NCORES):
        shard = np.ascontiguousarray(
            bankf[:, i * NSH:(i + 1) * NSH, :]).reshape(ROWS, 1024)
        in_maps.append({"smalls": sm, "bank": shard})
    return in_maps


_LAST_RESULTS = None  # stashed for test.py introspection


def kernel(hazards, S, indiv, gene, path, cohort_bank, label, c):
    global _LAST_RESULTS
    nc = _get_nc()
    in_maps = _make_in_maps(hazards, S, indiv, gene, path, cohort_bank, label, c)
    trace = bool(int(os.environ.get("TRNK_TRACE", "0")))
    res = run_bass_kernel_spmd(nc, in_maps, core_ids=list(range(NCORES)),
                               trace=trace)
    _LAST_RESULTS = res
    outs = np.stack([r["out_vec"][:, 0] for r in res.results])  # [8, 4]
    ep = float(outs[:, 0].sum())
    en = float(outs[:, 1].sum())
    other = float(outs[:, 2].mean())
    loss = other - math.log((ep + EPS_COH) / (ep + en + EPS_COH))
    return np.float32(loss)
